# revision 12
# baseline (speedup 1.0000x reference)
"""Trainium2 Bass kernel for AdaptiveAttentionTransformerBlock (sparse attention).

Self-contained: hardcodes shapes/sharding. Sequence-sharded across 8 cores
(2 batches x 4 sequence slices of 512 tokens); no collectives needed because
the attention mask (block-local 256 | sliding window 128 | 4 global tokens,
causal) only requires a 128-token halo plus the 4 global tokens per slice.

Per-core pipeline (all matmuls bf16 inputs -> f32 PSUM accum):
  xT [1024, 644] (halo 128 | own 512 | global 4) feature-major
  Q/K proj -> feature-major [feat, tok]; V proj -> token-major [tok, feat]
  RoPE via const matmul R (rotate_half) + elementwise cos/sin (f32)
  scoresT [k, q] computed directly (no attn transpose needed for AV)
  exp without max-subtraction (logits are O(5) for this data), masks are
  0/1 multiplies after exp, softmax denominator via an appended ones-column
  in V (row 64 of the AV output), normalization by reciprocal broadcast
  Out proj from feature-major attn output, y [512, 1024] f32 per core.
"""
import sys

sys.path.insert(0, "/opt/trn_rl_repo")

import numpy as np
import ml_dtypes

import concourse.bacc as bacc
import concourse.bass as bass
import concourse.mybir as mybir
import concourse.tile as tile
from concourse import bass_utils

BF16 = ml_dtypes.bfloat16
F32 = mybir.dt.float32
BF = mybir.dt.bfloat16

EMB, HEADS, HD = 1024, 16, 64
B, S = 2, 2048
SCALE = HD ** -0.5
CTX = 644  # 128 halo + 512 own + 4 global
MUL = mybir.AluOpType.mult
ADD = mybir.AluOpType.add
EXP = mybir.ActivationFunctionType.Exp


def _build_graph(dbg=False):
    nc = bacc.Bacc("TRN2", target_bir_lowering=False, debug=False)

    D = {}
    D["xt"] = nc.dram_tensor("xt", [EMB, CTX], BF, kind="ExternalInput")
    for w in ("wq", "wk", "wv", "wo"):
        D[w] = nc.dram_tensor(w, [EMB, EMB], BF, kind="ExternalInput")
    D["cosq"] = nc.dram_tensor("cosq", [128, 512], F32, kind="ExternalInput")
    D["sinq"] = nc.dram_tensor("sinq", [128, 512], F32, kind="ExternalInput")
    D["cosk"] = nc.dram_tensor("cosk", [128, CTX], F32, kind="ExternalInput")
    D["sink"] = nc.dram_tensor("sink", [128, CTX], F32, kind="ExternalInput")
    D["rmat"] = nc.dram_tensor("rmat", [128, 128], BF, kind="ExternalInput")
    D["mtri"] = nc.dram_tensor("mtri", [128, 128], BF, kind="ExternalInput")
    D["mwin"] = nc.dram_tensor("mwin", [128, 128], BF, kind="ExternalInput")
    D["m0"] = nc.dram_tensor("m0", [128, 128], BF, kind="ExternalInput")
    D["gmask"] = nc.dram_tensor("gmask", [4, 128], BF, kind="ExternalInput")
    D["out"] = nc.dram_tensor("out", [512, EMB], F32, kind="ExternalOutput")
    if dbg:
        D["dbg_qrot0"] = nc.dram_tensor("dbg_qrot0", [128, 512], BF, kind="ExternalOutput")
        D["dbg_krot0"] = nc.dram_tensor("dbg_krot0", [128, CTX], BF, kind="ExternalOutput")
        D["dbg_vsb1"] = nc.dram_tensor("dbg_vsb1", [128, 16, 65], BF, kind="ExternalOutput")
        D["dbg_aT0"] = nc.dram_tensor("dbg_aT0", [128, 512], BF, kind="ExternalOutput")

    with tile.TileContext(nc) as tc:
        _body(nc, tc, D, dbg=dbg)

    nc.compile()
    return nc


def _body(nc, tc, D, dbg=False):
    from contextlib import ExitStack
    es = ExitStack()
    cp = es.enter_context(tc.tile_pool(name="const", bufs=1))
    wp = es.enter_context(tc.tile_pool(name="work", bufs=6, space=bass.MemorySpace.PSUM))
    avp = es.enter_context(tc.tile_pool(name="avp", bufs=2, space=bass.MemorySpace.PSUM))
    sp = es.enter_context(tc.tile_pool(name="sb", bufs=3))
    atp = es.enter_context(tc.tile_pool(name="att", bufs=4))

    # ---- persistent SBUF tiles ----
    xt = [cp.tile([128, CTX], BF, tag=f"xt{i}", name=f"xt{i}") for i in range(8)]
    wq = [cp.tile([128, EMB], BF, tag=f"wq{i}", name=f"wq{i}") for i in range(8)]
    wk = [cp.tile([128, EMB], BF, tag=f"wk{i}", name=f"wk{i}") for i in range(8)]
    wv = [cp.tile([128, EMB], BF, tag=f"wv{i}", name=f"wv{i}") for i in range(8)]
    wo = [cp.tile([128, EMB], BF, tag=f"wo{i}", name=f"wo{i}") for i in range(8)]
    cosq = cp.tile([128, 512], F32, tag="cosq")
    sinq = cp.tile([128, 512], F32, tag="sinq")
    cosk = cp.tile([128, CTX], F32, tag="cosk")
    sink = cp.tile([128, CTX], F32, tag="sink")
    rmat = cp.tile([128, 128], BF, tag="rmat")
    mtri = cp.tile([128, 128], BF, tag="mtri")
    mwin = cp.tile([128, 128], BF, tag="mwin")
    m0 = cp.tile([128, 128], BF, tag="m0")
    gmask = cp.tile([4, 128], BF, tag="gmask")
    zbias = cp.tile([128, 1], F32, tag="zbias")
    qrot = [cp.tile([128, 512], BF, tag=f"qrot{i}", name=f"qrot{i}") for i in range(8)]
    krot = [cp.tile([128, CTX], BF, tag=f"krot{i}", name=f"krot{i}") for i in range(8)]
    vsb = [cp.tile([128, 16, 65], BF, tag=f"vsb{i}", name=f"vsb{i}") for i in range(6)]
    aT = [cp.tile([128, 512], BF, tag=f"aT{i}", name=f"aT{i}") for i in range(8)]
    ysb = [cp.tile([128, EMB], F32, tag=f"ysb{i}", name=f"ysb{i}") for i in range(4)]

    # ---- DMA loads (spread across engines for parallel queues) ----
    engs = [nc.sync, nc.scalar, nc.gpsimd]
    di = 0

    def dma(dst, src):
        nonlocal di
        engs[di % len(engs)].dma_start(out=dst, in_=src)
        di += 1

    for i in range(8):
        dma(xt[i][:], D["xt"][128 * i:128 * (i + 1), :])
    for tiles, name in ((wq, "wq"), (wk, "wk"), (wv, "wv"), (wo, "wo")):
        for i in range(8):
            dma(tiles[i][:], D[name][128 * i:128 * (i + 1), :])
    for t, name in ((cosq, "cosq"), (sinq, "sinq"), (cosk, "cosk"), (sink, "sink"),
                    (rmat, "rmat"), (mtri, "mtri"), (mwin, "mwin"), (m0, "m0"),
                    (gmask, "gmask")):
        dma(t[:], D[name][:])

    nc.gpsimd.memset(zbias[:], 0.0)
    for t in range(6):
        nc.gpsimd.memset(vsb[t][:, :, 64:65], 1.0)

    # ---- Q/K projections + RoPE (feature-major [feat, tok]) ----
    def proj_rope(hp, wtiles, col0, cw, ctile, stile, rot_out):
        """project feature chunk hp over token cols [col0, col0+cw), rope it"""
        pp = wp.tile([128, 512], F32, tag="work")
        for e in range(8):
            nc.tensor.matmul(pp[:, 0:cw], lhsT=wtiles[e][:, hp * 128:(hp + 1) * 128],
                             rhs=xt[e][:, col0:col0 + cw],
                             start=(e == 0), stop=(e == 7))
        psb = sp.tile([128, 512], BF, tag="qsb")
        nc.vector.tensor_copy(psb[:, 0:cw], pp[:, 0:cw])
        rq = wp.tile([128, 512], F32, tag="work")
        nc.tensor.matmul(rq[:, 0:cw], lhsT=rmat[:], rhs=psb[:, 0:cw],
                         start=True, stop=True)
        t0 = sp.tile([128, 512], F32, tag="t0")
        nc.vector.tensor_tensor(out=t0[:, 0:cw], in0=pp[:, 0:cw],
                                in1=ctile[:, col0:col0 + cw], op=MUL)
        t1 = sp.tile([128, 512], F32, tag="t1")
        nc.vector.tensor_tensor(out=t1[:, 0:cw], in0=rq[:, 0:cw],
                                in1=stile[:, col0:col0 + cw], op=MUL)
        nc.vector.tensor_tensor(out=rot_out, in0=t0[:, 0:cw], in1=t1[:, 0:cw], op=ADD)

    for hp in range(8):
        # q: own tokens are xt cols 128:640; cosq indexed 0:512
        pp = wp.tile([128, 512], F32, tag="work")
        for e in range(8):
            nc.tensor.matmul(pp[:], lhsT=wq[e][:, hp * 128:(hp + 1) * 128],
                             rhs=xt[e][:, 128:640], start=(e == 0), stop=(e == 7))
        psb = sp.tile([128, 512], BF, tag="qsb")
        nc.vector.tensor_copy(psb[:], pp[:])
        rq = wp.tile([128, 512], F32, tag="work")
        nc.tensor.matmul(rq[:], lhsT=rmat[:], rhs=psb[:], start=True, stop=True)
        t0 = sp.tile([128, 512], F32, tag="t0")
        nc.vector.tensor_tensor(out=t0[:], in0=pp[:], in1=cosq[:], op=MUL)
        t1 = sp.tile([128, 512], F32, tag="t1")
        nc.vector.tensor_tensor(out=t1[:], in0=rq[:], in1=sinq[:], op=MUL)
        nc.vector.tensor_tensor(out=qrot[hp][:], in0=t0[:], in1=t1[:], op=ADD)
        # k: full context in two col chunks
        for (c0, cw) in ((0, 512), (512, 132)):
            proj_rope(hp, wk, c0, cw, cosk, sink, krot[hp][:, c0:c0 + cw])

    # ---- V projection (token-major [tok, vfeat]) ----
    for t in range(6):
        tok0, tw = (t * 128, 128) if t < 5 else (640, 4)
        for half in range(2):
            vp = wp.tile([128, 512], F32, tag="work")
            for e in range(8):
                nc.tensor.matmul(vp[0:tw, :], lhsT=xt[e][:, tok0:tok0 + tw],
                                 rhs=wv[e][:, half * 512:(half + 1) * 512],
                                 start=(e == 0), stop=(e == 7))
            nc.vector.tensor_copy(
                vsb[t][0:tw, half * 8:(half + 1) * 8, 0:64],
                vp[0:tw, :].rearrange("p (h d) -> p h d", h=8))

    # ---- attention: per (head, q-tile) ----
    for h in range(HEADS):
        hp, po = h // 2, (h % 2) * 64
        dsl = slice(po, po + 64)
        for Ic in range(4):
            q_ap = qrot[hp][dsl, Ic * 128:(Ic + 1) * 128]          # [64, 128]
            st = wp.tile([128, 384], F32, tag="work")
            nc.tensor.matmul(st[:, 0:128], lhsT=krot[hp][dsl, Ic * 128:Ic * 128 + 128],
                             rhs=q_ap, start=True, stop=True)
            nc.tensor.matmul(st[:, 128:256],
                             lhsT=krot[hp][dsl, 128 + Ic * 128:256 + Ic * 128],
                             rhs=q_ap, start=True, stop=True)
            nc.tensor.matmul(st[0:4, 256:384], lhsT=krot[hp][dsl, 640:644],
                             rhs=q_ap, start=True, stop=True)
            att = atp.tile([128, 256], BF, tag="att")
            nc.scalar.activation(att[:], st[:, 0:256], EXP, bias=zbias[:])
            ag = atp.tile([4, 128], BF, tag="ag")
            nc.scalar.activation(ag[:], st[0:4, 256:384], EXP, bias=zbias[0:4])
            # masks (0/1 bf16 multiplies, on gpsimd: SBUF only)
            nc.gpsimd.tensor_tensor(out=att[:, 128:256], in0=att[:, 128:256],
                                    in1=mtri[:], op=MUL)
            if Ic == 0:
                nc.gpsimd.tensor_tensor(out=att[:, 0:128], in0=att[:, 0:128],
                                        in1=m0[:], op=MUL)
            elif Ic == 2:
                nc.gpsimd.tensor_tensor(out=att[:, 0:128], in0=att[:, 0:128],
                                        in1=mwin[:], op=MUL)
            if Ic in (0, 1):
                nc.gpsimd.tensor_tensor(out=ag[:], in0=ag[:], in1=gmask[:], op=MUL)
            av = avp.tile([65, 128], F32, tag="av")
            nc.tensor.matmul(av[:], lhsT=vsb[Ic][:, h, 0:65], rhs=att[:, 0:128],
                             start=True, stop=False)
            nc.tensor.matmul(av[:], lhsT=vsb[Ic + 1][:, h, 0:65], rhs=att[:, 128:256],
                             start=False, stop=False)
            nc.tensor.matmul(av[:], lhsT=vsb[5][0:4, h, 0:65], rhs=ag[:],
                             start=False, stop=True)
            rec = sp.tile([1, 128], F32, tag="rec")
            nc.vector.reciprocal(out=rec[:], in_=av[64:65, :])
            rbc = sp.tile([64, 128], F32, tag="rbc")
            nc.gpsimd.partition_broadcast(rbc[:], rec[:])
            nc.vector.tensor_tensor(
                out=aT[hp][dsl, Ic * 128:(Ic + 1) * 128], in0=av[0:64, :],
                in1=rbc[:], op=MUL)

    # ---- out projection ----
    for Ic in range(4):
        for half in range(2):
            yp = wp.tile([128, 512], F32, tag="work")
            for fc in range(8):
                nc.tensor.matmul(yp[:], lhsT=aT[fc][:, Ic * 128:(Ic + 1) * 128],
                                 rhs=wo[fc][:, half * 512:(half + 1) * 512],
                                 start=(fc == 0), stop=(fc == 7))
            nc.vector.tensor_copy(ysb[Ic][:, half * 512:(half + 1) * 512], yp[:])
        nc.sync.dma_start(out=D["out"][Ic * 128:(Ic + 1) * 128, :], in_=ysb[Ic][:])

    if dbg:
        nc.sync.dma_start(out=D["dbg_qrot0"][:], in_=qrot[0][:])
        nc.sync.dma_start(out=D["dbg_krot0"][:], in_=krot[0][:])
        nc.sync.dma_start(out=D["dbg_vsb1"][:], in_=vsb[1][:])
        nc.sync.dma_start(out=D["dbg_aT0"][:], in_=aT[0][:])

    es.close()


# ---------------- host side ----------------

def _make_consts():
    inv_freq = 1.0 / (10000.0 ** (np.arange(0, HD, 2, dtype=np.float64) / HD))
    pos = np.arange(S, dtype=np.float64)
    freqs = np.outer(pos, inv_freq)
    emb = np.concatenate([freqs, freqs], -1)
    return np.cos(emb).astype(np.float32), np.sin(emb).astype(np.float32)


def _rmat2():
    R = np.zeros((HD, HD), np.float32)
    for i in range(HD // 2):
        R[2 * i, 2 * i + 1] = -1.0
        R[2 * i + 1, 2 * i] = 1.0
    R2 = np.zeros((128, 128), np.float32)
    R2[0:64, 0:64] = R
    R2[64:128, 64:128] = R
    return np.ascontiguousarray(R2.T)  # lhsT so that lhsT.T @ q = R2 @ q


def build_in_maps(x, qkv_w, out_w):
    x = np.asarray(x, np.float32)
    qkv_w = np.asarray(qkv_w, np.float32)
    out_w = np.asarray(out_w, np.float32)
    cos_full, sin_full = _make_consts()

    wq = np.ascontiguousarray(qkv_w[0:EMB].T).astype(BF16)
    wk = np.ascontiguousarray(qkv_w[EMB:2 * EMB].T).astype(BF16)
    wv = np.ascontiguousarray(qkv_w[2 * EMB:3 * EMB].T).astype(BF16)
    wo = np.ascontiguousarray(out_w.T).astype(BF16)
    rmat = _rmat2().astype(BF16)
    ar = np.arange(128)
    mtri = (ar[:, None] <= ar[None, :]).astype(np.float32).astype(BF16)
    mwin = (ar[:, None] >= ar[None, :]).astype(np.float32).astype(BF16)

    in_maps = []
    for c in range(8):
        b, si = c // 4, c % 4
        xb = x[b]
        ctx = np.zeros((CTX, EMB), np.float32)
        if si > 0:
            ctx[0:128] = xb[512 * si - 128:512 * si]
        ctx[128:640] = xb[512 * si:512 * si + 512]
        ctx[640:644] = xb[0:4]
        xt = np.ascontiguousarray(ctx.T).astype(BF16)

        own_pos = np.arange(512 * si, 512 * si + 512)
        ctx_pos = np.zeros(CTX, np.int64)
        if si > 0:
            ctx_pos[0:128] = np.arange(512 * si - 128, 512 * si)
        ctx_pos[128:640] = own_pos
        ctx_pos[640:644] = np.arange(4)

        cosq = np.ascontiguousarray(np.tile(cos_full[own_pos].T, (2, 1)) * SCALE)
        sinq = np.ascontiguousarray(np.tile(sin_full[own_pos].T, (2, 1)) * SCALE)
        cosk = np.ascontiguousarray(np.tile(cos_full[ctx_pos].T, (2, 1)))
        sink = np.ascontiguousarray(np.tile(sin_full[ctx_pos].T, (2, 1)))

        m0 = mwin if si > 0 else np.zeros((128, 128), BF16)
        gmask = np.full((4, 128), 1.0 if si > 0 else 0.0, np.float32).astype(BF16)

        in_maps.append({
            "xt": xt, "wq": wq, "wk": wk, "wv": wv, "wo": wo,
            "cosq": cosq, "sinq": sinq, "cosk": cosk, "sink": sink,
            "rmat": rmat, "mtri": mtri, "mwin": mwin, "m0": m0, "gmask": gmask,
        })
    return in_maps


_NC = None


def _get_nc():
    global _NC
    if _NC is None:
        _NC = _build_graph()
    return _NC


LAST_EXEC_NS = None
LAST_RESULTS = None


def _ensure_ntff_hook():
    """The image's antenv lacks axon_hooks; shim it so trace=True works."""
    import types
    try:
        import antenv.axon_hooks  # noqa: F401
        return
    except ImportError:
        pass
    import antenv
    mod = types.ModuleType("antenv.axon_hooks")
    state = {"hook": None}
    mod.set_axon_ntff_profile_hook = lambda h: state.__setitem__("hook", h)
    mod.get_axon_ntff_profile_hook = lambda: state["hook"]
    sys.modules["antenv.axon_hooks"] = mod
    antenv.axon_hooks = mod
    try:
        from trn_agent_boot.trn_boot import _ntff_profile_via_ctypes
        h = _ntff_profile_via_ctypes("/opt/axon/libaxon_pjrt.so")
        if h is not None:
            mod.set_axon_ntff_profile_hook(h)
    except Exception:
        pass


def _run(x, qkv_w, out_w, trace=False):
    global LAST_EXEC_NS, LAST_RESULTS
    if trace:
        _ensure_ntff_hook()
    nc = _get_nc()
    in_maps = build_in_maps(x, qkv_w, out_w)
    res = bass_utils.run_bass_kernel_spmd(nc, in_maps, core_ids=list(range(8)),
                                          trace=trace)
    LAST_EXEC_NS = res.exec_time_ns
    LAST_RESULTS = res
    y = np.zeros((B, S, EMB), np.float32)
    for c in range(8):
        b, si = c // 4, c % 4
        y[b, 512 * si:512 * si + 512] = res.results[c]["out"]
    return y


def kernel(x, qkv_w, out_w):
    return _run(x, qkv_w, out_w, trace=False)


# revision 13
# speedup vs baseline: 4.7339x; 4.7339x over previous
"""Trainium2 Bass kernel for AdaptiveAttentionTransformerBlock (sparse attention).

Self-contained: hardcodes shapes/sharding. Sequence-sharded across 8 cores
(2 batches x 4 sequence slices of 512 tokens); no collectives needed because
the attention mask (block-local 256 | sliding window 128 | 4 global tokens,
causal) only requires a 128-token halo plus the 4 global tokens per slice.

Per-core pipeline (all matmuls bf16 inputs -> f32 PSUM accum):
  xT [1024, 644] (halo 128 | own 512 | global 4) feature-major
  Q/K proj -> feature-major [feat, tok]; V proj -> token-major [tok, feat]
  RoPE via const matmul R (rotate_half) + elementwise cos/sin (f32)
  scoresT [k, q] computed directly (no attn transpose needed for AV)
  exp without max-subtraction (logits are O(5) for this data), masks are
  0/1 multiplies after exp, softmax denominator via an appended ones-column
  in V (row 64 of the AV output), normalization by reciprocal broadcast
  Out proj from feature-major attn output, y [512, 1024] f32 per core.
"""
import sys

sys.path.insert(0, "/opt/trn_rl_repo")

import numpy as np
import ml_dtypes

import concourse.bacc as bacc
import concourse.bass as bass
import concourse.mybir as mybir
import concourse.tile as tile
from concourse import bass_utils

BF16 = ml_dtypes.bfloat16
F32 = mybir.dt.float32
BF = mybir.dt.bfloat16

EMB, HEADS, HD = 1024, 16, 64
B, S = 2, 2048
SCALE = HD ** -0.5
CTX = 644  # 128 halo + 512 own + 4 global
MUL = mybir.AluOpType.mult
ADD = mybir.AluOpType.add
EXP = mybir.ActivationFunctionType.Exp


def _build_graph(dbg=False):
    nc = bacc.Bacc("TRN2", target_bir_lowering=False, debug=False)

    D = {}
    D["xt"] = nc.dram_tensor("xt", [EMB, CTX], BF, kind="ExternalInput")
    for w in ("wq", "wk", "wv", "wo"):
        D[w] = nc.dram_tensor(w, [EMB, EMB], BF, kind="ExternalInput")
    D["cosq"] = nc.dram_tensor("cosq", [128, 512], F32, kind="ExternalInput")
    D["sinq"] = nc.dram_tensor("sinq", [128, 512], F32, kind="ExternalInput")
    D["cosk"] = nc.dram_tensor("cosk", [128, CTX], F32, kind="ExternalInput")
    D["sink"] = nc.dram_tensor("sink", [128, CTX], F32, kind="ExternalInput")
    D["rmat"] = nc.dram_tensor("rmat", [128, 128], BF, kind="ExternalInput")
    D["mtri"] = nc.dram_tensor("mtri", [128, 128], BF, kind="ExternalInput")
    D["mwin"] = nc.dram_tensor("mwin", [128, 128], BF, kind="ExternalInput")
    D["m0"] = nc.dram_tensor("m0", [128, 128], BF, kind="ExternalInput")
    D["gmask"] = nc.dram_tensor("gmask", [4, 128], BF, kind="ExternalInput")
    D["out"] = nc.dram_tensor("out", [512, EMB], F32, kind="ExternalOutput")
    if dbg:
        D["dbg_qrot0"] = nc.dram_tensor("dbg_qrot0", [128, 512], BF, kind="ExternalOutput")
        D["dbg_krot0"] = nc.dram_tensor("dbg_krot0", [128, CTX], BF, kind="ExternalOutput")
        D["dbg_vsb1"] = nc.dram_tensor("dbg_vsb1", [128, 16, 65], BF, kind="ExternalOutput")
        D["dbg_aT0"] = nc.dram_tensor("dbg_aT0", [128, 512], BF, kind="ExternalOutput")

    with tile.TileContext(nc) as tc:
        _body(nc, tc, D, dbg=dbg)

    nc.compile()
    return nc


def _body(nc, tc, D, dbg=False):
    from contextlib import ExitStack
    es = ExitStack()
    cp = es.enter_context(tc.tile_pool(name="const", bufs=1))
    wp = es.enter_context(tc.tile_pool(name="work", bufs=5, space=bass.MemorySpace.PSUM))
    avp = es.enter_context(tc.tile_pool(name="avp", bufs=3, space=bass.MemorySpace.PSUM))
    sp = es.enter_context(tc.tile_pool(name="sb", bufs=4))
    atp = es.enter_context(tc.tile_pool(name="att", bufs=6))

    # ---- persistent SBUF tiles ----
    xt = [cp.tile([128, CTX], BF, tag=f"xt{i}", name=f"xt{i}") for i in range(8)]
    wq = [cp.tile([128, EMB], BF, tag=f"wq{i}", name=f"wq{i}") for i in range(8)]
    wk = [cp.tile([128, EMB], BF, tag=f"wk{i}", name=f"wk{i}") for i in range(8)]
    wv = [cp.tile([128, EMB], BF, tag=f"wv{i}", name=f"wv{i}") for i in range(8)]
    wo = [cp.tile([128, EMB], BF, tag=f"wo{i}", name=f"wo{i}") for i in range(8)]
    cosq = cp.tile([128, 512], F32, tag="cosq")
    sinq = cp.tile([128, 512], F32, tag="sinq")
    cosk = cp.tile([128, CTX], F32, tag="cosk")
    sink = cp.tile([128, CTX], F32, tag="sink")
    rmat = cp.tile([128, 128], BF, tag="rmat")
    mtri = cp.tile([128, 128], BF, tag="mtri")
    mwin = cp.tile([128, 128], BF, tag="mwin")
    m0 = cp.tile([128, 128], BF, tag="m0")
    gmask = cp.tile([4, 128], BF, tag="gmask")
    zbias = cp.tile([128, 1], F32, tag="zbias")
    qrot = [cp.tile([128, 512], BF, tag=f"qrot{i}", name=f"qrot{i}") for i in range(8)]
    krot = [cp.tile([128, CTX], BF, tag=f"krot{i}", name=f"krot{i}") for i in range(8)]
    vsb = [cp.tile([128, 16, 65], BF, tag=f"vsb{i}", name=f"vsb{i}") for i in range(6)]
    aT = [cp.tile([128, 512], BF, tag=f"aT{i}", name=f"aT{i}") for i in range(8)]
    ysb = [cp.tile([128, EMB], F32, tag=f"ysb{i}", name=f"ysb{i}") for i in range(4)]

    # ---- DMA loads (spread across engines for parallel queues) ----
    engs = [nc.sync, nc.scalar]
    di = 0

    def dma(dst, src):
        nonlocal di
        engs[di % len(engs)].dma_start(out=dst, in_=src)
        di += 1

    for i in range(8):
        dma(xt[i][:], D["xt"][128 * i:128 * (i + 1), :])
    for tiles, name in ((wq, "wq"), (wk, "wk"), (wv, "wv"), (wo, "wo")):
        for i in range(8):
            dma(tiles[i][:], D[name][128 * i:128 * (i + 1), :])
    for t, name in ((cosq, "cosq"), (sinq, "sinq"), (cosk, "cosk"), (sink, "sink"),
                    (rmat, "rmat"), (mtri, "mtri"), (mwin, "mwin"), (m0, "m0"),
                    (gmask, "gmask")):
        dma(t[:], D[name][:])

    nc.vector.memset(zbias[:], 0.0)
    for t in range(6):
        nc.vector.memset(vsb[t][:, :, 64:65], 1.0)

    # ---- Q/K projections + RoPE (feature-major [feat, tok]) ----
    def proj_rope(hp, wtiles, col0, cw, ctile, stile, rot_out):
        """project feature chunk hp over token cols [col0, col0+cw), rope it"""
        pp = wp.tile([128, 512], F32, tag="work")
        for e in range(8):
            nc.tensor.matmul(pp[:, 0:cw], lhsT=wtiles[e][:, hp * 128:(hp + 1) * 128],
                             rhs=xt[e][:, col0:col0 + cw],
                             start=(e == 0), stop=(e == 7))
        psb = sp.tile([128, 512], BF, tag="qsb")
        nc.vector.tensor_copy(psb[:, 0:cw], pp[:, 0:cw])
        rq = wp.tile([128, 512], F32, tag="work")
        nc.tensor.matmul(rq[:, 0:cw], lhsT=rmat[:], rhs=psb[:, 0:cw],
                         start=True, stop=True)
        t0 = sp.tile([128, 512], F32, tag="t0")
        nc.vector.tensor_tensor(out=t0[:, 0:cw], in0=pp[:, 0:cw],
                                in1=ctile[:, col0:col0 + cw], op=MUL)
        t1 = sp.tile([128, 512], F32, tag="t1")
        nc.vector.tensor_tensor(out=t1[:, 0:cw], in0=rq[:, 0:cw],
                                in1=stile[:, col0:col0 + cw], op=MUL)
        nc.vector.tensor_tensor(out=rot_out, in0=t0[:, 0:cw], in1=t1[:, 0:cw], op=ADD)

    for hp in range(8):
        # q: own tokens are xt cols 128:640; cosq indexed 0:512
        pp = wp.tile([128, 512], F32, tag="work")
        for e in range(8):
            nc.tensor.matmul(pp[:], lhsT=wq[e][:, hp * 128:(hp + 1) * 128],
                             rhs=xt[e][:, 128:640], start=(e == 0), stop=(e == 7))
        psb = sp.tile([128, 512], BF, tag="qsb")
        nc.vector.tensor_copy(psb[:], pp[:])
        rq = wp.tile([128, 512], F32, tag="work")
        nc.tensor.matmul(rq[:], lhsT=rmat[:], rhs=psb[:], start=True, stop=True)
        t0 = sp.tile([128, 512], F32, tag="t0")
        nc.vector.tensor_tensor(out=t0[:], in0=pp[:], in1=cosq[:], op=MUL)
        t1 = sp.tile([128, 512], F32, tag="t1")
        nc.vector.tensor_tensor(out=t1[:], in0=rq[:], in1=sinq[:], op=MUL)
        nc.vector.tensor_tensor(out=qrot[hp][:], in0=t0[:], in1=t1[:], op=ADD)
        # k: full context in two col chunks
        for (c0, cw) in ((0, 512), (512, 132)):
            proj_rope(hp, wk, c0, cw, cosk, sink, krot[hp][:, c0:c0 + cw])

    # ---- V projection (token-major [tok, vfeat]) ----
    for t in range(6):
        tok0, tw = (t * 128, 128) if t < 5 else (640, 4)
        for half in range(2):
            vp = wp.tile([128, 512], F32, tag="work")
            for e in range(8):
                nc.tensor.matmul(vp[0:tw, :], lhsT=xt[e][:, tok0:tok0 + tw],
                                 rhs=wv[e][:, half * 512:(half + 1) * 512],
                                 start=(e == 0), stop=(e == 7))
            nc.vector.tensor_copy(
                vsb[t][0:tw, half * 8:(half + 1) * 8, 0:64],
                vp[0:tw, :].rearrange("p (h d) -> p h d", h=8))

    # ---- attention: per (head, q-tile) ----
    for h in range(HEADS):
        hp, po = h // 2, (h % 2) * 64
        dsl = slice(po, po + 64)
        for Ic in range(4):
            q_ap = qrot[hp][dsl, Ic * 128:(Ic + 1) * 128]          # [64, 128]
            st = wp.tile([128, 384], F32, tag="work")
            nc.tensor.matmul(st[:, 0:128], lhsT=krot[hp][dsl, Ic * 128:Ic * 128 + 128],
                             rhs=q_ap, start=True, stop=True)
            nc.tensor.matmul(st[:, 128:256],
                             lhsT=krot[hp][dsl, 128 + Ic * 128:256 + Ic * 128],
                             rhs=q_ap, start=True, stop=True)
            nc.tensor.matmul(st[0:4, 256:384], lhsT=krot[hp][dsl, 640:644],
                             rhs=q_ap, start=True, stop=True)
            att = atp.tile([128, 384], BF, tag="att")
            nc.scalar.activation(att[:], st[:], EXP, bias=zbias[:])
            ag = att[0:4, 256:384]
            # masks (0/1 bf16 multiplies on DVE; gpsimd stays pinned to
            # partition_broadcast so its ucode library never reloads)
            nc.vector.tensor_tensor(out=att[:, 128:256], in0=att[:, 128:256],
                                    in1=mtri[:], op=MUL)
            if Ic == 0:
                nc.vector.tensor_tensor(out=att[:, 0:128], in0=att[:, 0:128],
                                        in1=m0[:], op=MUL)
            elif Ic == 2:
                nc.vector.tensor_tensor(out=att[:, 0:128], in0=att[:, 0:128],
                                        in1=mwin[:], op=MUL)
            if Ic in (0, 1):
                nc.vector.tensor_tensor(out=ag, in0=ag, in1=gmask[:], op=MUL)
            av = avp.tile([65, 128], F32, tag="av")
            nc.tensor.matmul(av[:], lhsT=vsb[Ic][:, h, 0:65], rhs=att[:, 0:128],
                             start=True, stop=False)
            nc.tensor.matmul(av[:], lhsT=vsb[Ic + 1][:, h, 0:65], rhs=att[:, 128:256],
                             start=False, stop=False)
            nc.tensor.matmul(av[:], lhsT=vsb[5][0:4, h, 0:65], rhs=ag,
                             start=False, stop=True)
            rec = sp.tile([1, 128], F32, tag="rec")
            nc.vector.reciprocal(out=rec[:], in_=av[64:65, :])
            rbc = sp.tile([64, 128], F32, tag="rbc")
            nc.gpsimd.partition_broadcast(rbc[:], rec[:])
            nc.vector.tensor_tensor(
                out=aT[hp][dsl, Ic * 128:(Ic + 1) * 128], in0=av[0:64, :],
                in1=rbc[:], op=MUL)

    # ---- out projection ----
    for Ic in range(4):
        for half in range(2):
            yp = wp.tile([128, 512], F32, tag="work")
            for fc in range(8):
                nc.tensor.matmul(yp[:], lhsT=aT[fc][:, Ic * 128:(Ic + 1) * 128],
                                 rhs=wo[fc][:, half * 512:(half + 1) * 512],
                                 start=(fc == 0), stop=(fc == 7))
            nc.vector.tensor_copy(ysb[Ic][:, half * 512:(half + 1) * 512], yp[:])
        nc.sync.dma_start(out=D["out"][Ic * 128:(Ic + 1) * 128, :], in_=ysb[Ic][:])

    if dbg:
        nc.sync.dma_start(out=D["dbg_qrot0"][:], in_=qrot[0][:])
        nc.sync.dma_start(out=D["dbg_krot0"][:], in_=krot[0][:])
        nc.sync.dma_start(out=D["dbg_vsb1"][:], in_=vsb[1][:])
        nc.sync.dma_start(out=D["dbg_aT0"][:], in_=aT[0][:])

    es.close()


# ---------------- host side ----------------

def _make_consts():
    inv_freq = 1.0 / (10000.0 ** (np.arange(0, HD, 2, dtype=np.float64) / HD))
    pos = np.arange(S, dtype=np.float64)
    freqs = np.outer(pos, inv_freq)
    emb = np.concatenate([freqs, freqs], -1)
    return np.cos(emb).astype(np.float32), np.sin(emb).astype(np.float32)


def _rmat2():
    R = np.zeros((HD, HD), np.float32)
    for i in range(HD // 2):
        R[2 * i, 2 * i + 1] = -1.0
        R[2 * i + 1, 2 * i] = 1.0
    R2 = np.zeros((128, 128), np.float32)
    R2[0:64, 0:64] = R
    R2[64:128, 64:128] = R
    return np.ascontiguousarray(R2.T)  # lhsT so that lhsT.T @ q = R2 @ q


def build_in_maps(x, qkv_w, out_w):
    x = np.asarray(x, np.float32)
    qkv_w = np.asarray(qkv_w, np.float32)
    out_w = np.asarray(out_w, np.float32)
    cos_full, sin_full = _make_consts()

    wq = np.ascontiguousarray(qkv_w[0:EMB].T).astype(BF16)
    wk = np.ascontiguousarray(qkv_w[EMB:2 * EMB].T).astype(BF16)
    wv = np.ascontiguousarray(qkv_w[2 * EMB:3 * EMB].T).astype(BF16)
    wo = np.ascontiguousarray(out_w.T).astype(BF16)
    rmat = _rmat2().astype(BF16)
    ar = np.arange(128)
    mtri = (ar[:, None] <= ar[None, :]).astype(np.float32).astype(BF16)
    mwin = (ar[:, None] >= ar[None, :]).astype(np.float32).astype(BF16)

    in_maps = []
    for c in range(8):
        b, si = c // 4, c % 4
        xb = x[b]
        ctx = np.zeros((CTX, EMB), np.float32)
        if si > 0:
            ctx[0:128] = xb[512 * si - 128:512 * si]
        ctx[128:640] = xb[512 * si:512 * si + 512]
        ctx[640:644] = xb[0:4]
        xt = np.ascontiguousarray(ctx.T).astype(BF16)

        own_pos = np.arange(512 * si, 512 * si + 512)
        ctx_pos = np.zeros(CTX, np.int64)
        if si > 0:
            ctx_pos[0:128] = np.arange(512 * si - 128, 512 * si)
        ctx_pos[128:640] = own_pos
        ctx_pos[640:644] = np.arange(4)

        cosq = np.ascontiguousarray(np.tile(cos_full[own_pos].T, (2, 1)) * SCALE)
        sinq = np.ascontiguousarray(np.tile(sin_full[own_pos].T, (2, 1)) * SCALE)
        cosk = np.ascontiguousarray(np.tile(cos_full[ctx_pos].T, (2, 1)))
        sink = np.ascontiguousarray(np.tile(sin_full[ctx_pos].T, (2, 1)))

        m0 = mwin if si > 0 else np.zeros((128, 128), BF16)
        gmask = np.full((4, 128), 1.0 if si > 0 else 0.0, np.float32).astype(BF16)

        in_maps.append({
            "xt": xt, "wq": wq, "wk": wk, "wv": wv, "wo": wo,
            "cosq": cosq, "sinq": sinq, "cosk": cosk, "sink": sink,
            "rmat": rmat, "mtri": mtri, "mwin": mwin, "m0": m0, "gmask": gmask,
        })
    return in_maps


_NC = None


def _get_nc():
    global _NC
    if _NC is None:
        _NC = _build_graph()
    return _NC


LAST_EXEC_NS = None
LAST_RESULTS = None


def _ensure_ntff_hook():
    """The image's antenv lacks axon_hooks; shim it so trace=True works."""
    import types
    try:
        import antenv.axon_hooks  # noqa: F401
        return
    except ImportError:
        pass
    import antenv
    mod = types.ModuleType("antenv.axon_hooks")
    state = {"hook": None}
    mod.set_axon_ntff_profile_hook = lambda h: state.__setitem__("hook", h)
    mod.get_axon_ntff_profile_hook = lambda: state["hook"]
    sys.modules["antenv.axon_hooks"] = mod
    antenv.axon_hooks = mod
    try:
        from trn_agent_boot.trn_boot import _ntff_profile_via_ctypes
        h = _ntff_profile_via_ctypes("/opt/axon/libaxon_pjrt.so")
        if h is not None:
            mod.set_axon_ntff_profile_hook(h)
    except Exception:
        pass


def _run(x, qkv_w, out_w, trace=False):
    global LAST_EXEC_NS, LAST_RESULTS
    if trace:
        _ensure_ntff_hook()
    nc = _get_nc()
    in_maps = build_in_maps(x, qkv_w, out_w)
    res = bass_utils.run_bass_kernel_spmd(nc, in_maps, core_ids=list(range(8)),
                                          trace=trace)
    LAST_EXEC_NS = res.exec_time_ns
    LAST_RESULTS = res
    y = np.zeros((B, S, EMB), np.float32)
    for c in range(8):
        b, si = c // 4, c % 4
        y[b, 512 * si:512 * si + 512] = res.results[c]["out"]
    return y


def kernel(x, qkv_w, out_w):
    return _run(x, qkv_w, out_w, trace=False)
